# revision 26
# baseline (speedup 1.0000x reference)
"""Bass/Tile kernel for BilinearAttentionLayer on 8 NeuronCores.

out[b] = softmax(x[b] @ W @ x[b]^T / sqrt(D)) @ x[b]

Sharding: data-parallel over batch (8 batches -> 8 cores). Each core runs an
identical program on its own batch slice.

Per-core layout: keep the score matrix transposed (scoresT[t, s]) so every
matmul operand is naturally oriented and no score-matrix transposes are
needed:
  xwT   = matmul(lhsT=W,  rhs=xT)       [e, s]
  prodT = matmul(lhsT=xT, rhs=xwT)      [t, s]   (softmax axis = partitions)
  PT    = exp((prodT - rowmax)/sqrt(D))           (numerically safe softmax)
  rowsum= matmul(lhsT=ones, rhs=PT)     [1, s]
  out   = matmul(lhsT=PT, rhs=x_nat)    [s, d]   (lands natural)
The only data transposes are 64 PE transposes of x itself plus tiny
[1,512]<->[128,4] DRAM bounces for the per-row max / rowsum vectors.

Row max (softmax axis = partitions here) is computed as: DVE max-accumulate
across the 16 prodT tiles -> PE transpose of the [128, 512] accumulator ->
DVE free-axis reduce -> broadcast back across partitions with a K=1
ones-matmul -> DVE subtract (in place on the staged prodT strip) -> ScalarE
Exp in place (rounding to f32r for the PE).

Matmul operands are stored as float32r (1 cycle/row on the PE at N=512 vs 4
for float32); producers round to f32r as walrus requires.
"""

import numpy as np

import concourse.bass as bass
import concourse.mybir as mybir
import concourse.tile as tile
from concourse import bacc
from concourse import bass_isa
from concourse import bass_utils
from concourse.masks import make_identity

B = 8
S = 2048
D = 512
P = 128
SB = 512  # s-block width (one fp32 PSUM bank)

F32 = mybir.dt.float32
F32R = mybir.dt.float32r
BF16 = mybir.dt.bfloat16

SCALE = float(1.0 / np.sqrt(np.float64(D)))
AX = mybir.AxisListType
AF = mybir.ActivationFunctionType


def build_nc(s=S, d=D):
    nd = d // P   # d/e tiles of 128
    nst = s // P  # s/t tiles of 128
    nsb = s // SB  # s-blocks
    nss = SB // P  # 128-chunks per s-block

    nc = bacc.Bacc(
        "TRN2",
        target_bir_lowering=False,
        debug=False,
        num_devices=B,
    )
    x_d = nc.dram_tensor("x", [s, d], F32, kind="ExternalInput").ap()
    w_d = nc.dram_tensor("w", [d, d], F32, kind="ExternalInput").ap()
    o_d = nc.dram_tensor("o", [s, d], F32, kind="ExternalOutput").ap()

    x_tiled = x_d.rearrange("(n p) d -> p n d", p=P)  # [128, nst, d]
    w_tiled = w_d.rearrange("(k p) e -> p k e", p=P)  # [128, nd, d]
    o_tiled = o_d.rearrange("(n p) d -> p n d", p=P)

    with tile.TileContext(nc) as tc:
        with (
            tc.tile_pool(name="const", bufs=1) as constp,
            tc.tile_pool(name="big", bufs=1) as bigp,
            tc.tile_pool(name="stage", bufs=2) as stagep,
            tc.tile_pool(name="strip", bufs=2) as stripp,
            tc.tile_pool(name="ptp", bufs=2) as ptp,
            tc.tile_pool(name="bcast", bufs=2) as bcp,
            tc.tile_pool(name="outs", bufs=2) as outp,
            tc.tile_pool(name="acc", bufs=2) as accp,
            tc.tile_pool(name="small", bufs=2) as smallp,
            tc.tile_pool(name="dram", bufs=2, space="DRAM") as dramp,
            tc.tile_pool(name="mm", bufs=4, space="PSUM") as mmp,
            tc.tile_pool(name="tr", bufs=2, space="PSUM") as trp,
            tc.tile_pool(name="rs", bufs=2, space="PSUM") as rsp,
        ):
            ident = constp.tile([P, P], F32)
            make_identity(nc, ident[:])
            ones = constp.tile([P, 1], BF16)
            nc.vector.memset(ones[:], 1.0)
            x_nat = bigp.tile([P, nst, d], BF16)
            xT = bigp.tile([P, nd, s], F32R)
            w_sb = bigp.tile([P, nd, d], F32R)
            xwT = bigp.tile([P, nd, s], F32R)

            def load_w():
                for kt in range(nd):
                    wst = stagep.tile([P, d], F32, tag="st", name="wst")
                    nc.sync.dma_start(wst[:], w_tiled[:, kt, :])
                    nc.vector.tensor_copy(w_sb[:, kt, :], wst[:])

            def load_tile(st):
                xst = stagep.tile([P, d], F32, tag="st", name="xst")
                nc.sync.dma_start(xst[:], x_tiled[:, st, :])
                # round to bf16 for the PV matmul
                nc.vector.tensor_copy(x_nat[:, st, :], xst[:])
                # xT[p, dt, st*128+q] = x[st*128+q, dt*128+p]
                # 4 transposed blocks land in one PSUM bank -> single copy out
                ps = trp.tile([P, nd, P], F32, tag="tr", name="trps")
                for dt in range(nd):
                    nc.tensor.transpose(
                        ps[:, dt, :], xst[:, dt * P:(dt + 1) * P], ident[:]
                    )
                nc.vector.tensor_copy(xT[:, :, st * P:(st + 1) * P], ps[:])

            def xw_chunk(sb):
                # xwT[e, s-block] = sum_d W[d, e] x[s, d]
                for et in range(nd):
                    ps = mmp.tile([P, SB], F32, tag="mm", name="mmps")
                    for kt in range(nd):
                        nc.tensor.matmul(
                            ps[:],
                            w_sb[:, kt, et * P:(et + 1) * P],
                            xT[:, kt, sb * SB:(sb + 1) * SB],
                            start=(kt == 0),
                            stop=(kt == nd - 1),
                        )
                    nc.vector.tensor_copy(xwT[:, et, sb * SB:(sb + 1) * SB], ps[:])

            for sb in range(nsb):
                for st in range(nss * sb, nss * (sb + 1)):
                    load_tile(st)
                if sb == 0:
                    load_w()
                xw_chunk(sb)

            strips = [None] * nsb
            pts = [None] * nsb
            bcs = [None] * nsb

            def sub_exp(sb, tt):
                """shifted-exp of one staged tile (DVE sub + ScalarE exp),
                plus the zipped rowsum partial accumulate (DVE)."""
                strip = strips[sb]
                nc.gpsimd.tensor_sub(
                    strip[:, tt, :], strip[:, tt, :], bcs[sb][:]
                )
                nc.scalar.activation(
                    pts[sb][:, tt, :],
                    strip[:, tt, :],
                    AF.Exp,
                    scale=SCALE,
                )

            def prod_tiles(sb, prev):
                """prodT tiles of block sb (zipped with sub/exp of `prev` so
                no engine's static stream ever stalls)."""
                strip = stripp.tile([P, nst, SB], F32, tag="strip")
                pts[sb] = ptp.tile([P, nst, SB], BF16, tag="pt", name="pt")
                strips[sb] = strip
                acc = None
                for tt in range(nst):
                    ps = mmp.tile([P, SB], F32, tag="mm")
                    for et in range(nd):
                        nc.tensor.matmul(
                            ps[:],
                            xT[:, et, tt * P:(tt + 1) * P],
                            xwT[:, et, sb * SB:(sb + 1) * SB],
                            start=(et == 0),
                            stop=(et == nd - 1),
                        )
                    # stage raw scores (rounding to f32r, on ScalarE to keep
                    # DVE free) and max-accumulate on DVE
                    nc.scalar.copy(strip[:, tt, :], ps[:])
                    acc_new = accp.tile([P, SB], F32, tag="acc")
                    if acc is None:
                        nc.vector.tensor_copy(acc_new[:], ps[:])
                    else:
                        nc.vector.tensor_max(acc_new[:], ps[:], acc[:])
                    acc = acc_new
                    if prev is not None:
                        sub_exp(prev, tt)
                # row max, replicated across all partitions, on idle GpSimd
                bc = bcp.tile([P, SB], F32, tag="bc", name="bc", bufs=1)
                nc.gpsimd.partition_all_reduce(
                    bc[:], acc[:], channels=P, reduce_op=bass_isa.ReduceOp.max
                )
                bcs[sb] = bc

            def stage_b(sb, zipnext=None):
                """rowsum -> reciprocal -> PV matmul -> normalize -> store."""
                ptt = pts[sb]

                rs_ps = rsp.tile([1, SB], F32, tag="rs", name="rsps")
                for tt in range(nst):
                    nc.tensor.matmul(
                        rs_ps[:],
                        ones[:],
                        ptt[:, tt, :],
                        start=(tt == 0),
                        stop=(tt == nst - 1),
                    )
                    if zipnext is not None:
                        sub_exp(zipnext, tt)
                rs_row = smallp.tile([1, SB], F32, tag="rsrow", name="rsrow")
                nc.vector.tensor_copy(rs_row[:], rs_ps[:])
                rs_dram = dramp.tile([SB], F32, tag="rsdram", name="rsdram")
                nc.sync.dma_start(rs_dram[:], rs_row[:])
                rs_part = smallp.tile([P, nss], F32, tag="rspart", name="rspart")
                nc.sync.dma_start(
                    rs_part[:], rs_dram.rearrange("(f p) -> p f", p=P)
                )
                rs_rec = smallp.tile([P, nss], F32, tag="rsrec", name="rsrec")
                nc.vector.reciprocal(rs_rec[:], rs_part[:])

                # out[s, d] = sum_t P[s, t] x[t, d] ; lhsT = PT (already T!)
                for ss in range(nss):
                    ps = mmp.tile([P, d], F32, tag="mm", name="mmps")
                    for tt in range(nst):
                        nc.tensor.matmul(
                            ps[:],
                            ptt[:, tt, ss * P:(ss + 1) * P],
                            x_nat[:, tt, :],
                            start=(tt == 0),
                            stop=(tt == nst - 1),
                        )
                    ot = outp.tile([P, d], F32, tag="ot", name="ot")
                    nc.vector.tensor_scalar_mul(
                        ot[:], ps[:], rs_rec[:, ss:ss + 1]
                    )
                    nc.sync.dma_start(o_tiled[:, sb * nss + ss, :], ot[:])

            # software pipeline: block sb's sub/exp ops are zipped into the
            # next chunk of PE-heavy work (block sb+1's prodT tiles, or the
            # previous block's rowsum loop) so no static engine stream stalls
            # on the cross-engine max reduction.
            if nsb == 1:
                prod_tiles(0, None)
                for tt in range(nst):
                    sub_exp(0, tt)
                stage_b(0)
            else:
                prod_tiles(0, None)
                prod_tiles(1, 0)
                for sb in range(2, nsb):
                    stage_b(sb - 2)
                    prod_tiles(sb, sb - 1)
                stage_b(nsb - 2, zipnext=nsb - 1)
                stage_b(nsb - 1)

    nc.compile()
    return nc


_NC_CACHE = {}


def _get_nc():
    if "nc" not in _NC_CACHE:
        _NC_CACHE["nc"] = build_nc()
    return _NC_CACHE["nc"]


def kernel(x: np.ndarray, attn_matrix: np.ndarray) -> np.ndarray:
    assert x.shape == (B, S, D) and attn_matrix.shape == (D, D)
    nc = _get_nc()
    w = np.ascontiguousarray(attn_matrix, dtype=np.float32)
    in_maps = [
        {"x": np.ascontiguousarray(x[b], dtype=np.float32), "w": w}
        for b in range(B)
    ]
    res = bass_utils.run_bass_kernel_spmd(nc, in_maps, core_ids=list(range(B)))
    out = np.stack([res.results[b]["o"] for b in range(B)], axis=0)
    return out.astype(np.float32, copy=False)


# revision 27
# speedup vs baseline: 1.3865x; 1.3865x over previous
"""Bass/Tile kernel for BilinearAttentionLayer on 8 NeuronCores.

out[b] = softmax(x[b] @ W @ x[b]^T / sqrt(D)) @ x[b]

Sharding: data-parallel over batch (8 batches -> 8 cores). Each core runs an
identical program on its own batch slice.

Per-core layout: keep the score matrix transposed (scoresT[t, s]) so every
matmul operand is naturally oriented and no score-matrix transposes are
needed:
  xwT   = matmul(lhsT=W,  rhs=xT)       [e, s]
  prodT = matmul(lhsT=xT, rhs=xwT)      [t, s]   (softmax axis = partitions)
  PT    = exp((prodT - rowmax)/sqrt(D))           (numerically safe softmax)
  rowsum= matmul(lhsT=ones, rhs=PT)     [1, s]
  out   = matmul(lhsT=PT, rhs=x_nat)    [s, d]   (lands natural)
The only data transposes are 64 PE transposes of x itself plus tiny
[1,512]<->[128,4] DRAM bounces for the per-row max / rowsum vectors.

Row max (softmax axis = partitions here) is computed as: DVE max-accumulate
across the 16 prodT tiles -> PE transpose of the [128, 512] accumulator ->
DVE free-axis reduce -> broadcast back across partitions with a K=1
ones-matmul -> DVE subtract (in place on the staged prodT strip) -> ScalarE
Exp in place (rounding to f32r for the PE).

Matmul operands are stored as float32r (1 cycle/row on the PE at N=512 vs 4
for float32); producers round to f32r as walrus requires.
"""

import numpy as np

import concourse.bass as bass
import concourse.mybir as mybir
import concourse.tile as tile
from concourse import bacc
from concourse import bass_isa
from concourse import bass_utils
from concourse.masks import make_identity

B = 8
S = 2048
D = 512
P = 128
SB = 512  # s-block width (one fp32 PSUM bank)

F32 = mybir.dt.float32
F32R = mybir.dt.float32r
BF16 = mybir.dt.bfloat16

SCALE = float(1.0 / np.sqrt(np.float64(D)))
AX = mybir.AxisListType
AF = mybir.ActivationFunctionType


def build_nc(s=S, d=D):
    nd = d // P   # d/e tiles of 128
    nst = s // P  # s/t tiles of 128
    nsb = s // SB  # s-blocks
    nss = SB // P  # 128-chunks per s-block

    nc = bacc.Bacc(
        "TRN2",
        target_bir_lowering=False,
        debug=False,
        num_devices=B,
    )
    x_d = nc.dram_tensor("x", [s, d], F32, kind="ExternalInput").ap()
    w_d = nc.dram_tensor("w", [d, d], F32, kind="ExternalInput").ap()
    o_d = nc.dram_tensor("o", [s, d], F32, kind="ExternalOutput").ap()

    x_tiled = x_d.rearrange("(n p) d -> p n d", p=P)  # [128, nst, d]
    w_tiled = w_d.rearrange("(k p) e -> p k e", p=P)  # [128, nd, d]
    o_tiled = o_d.rearrange("(n p) d -> p n d", p=P)

    with tile.TileContext(nc) as tc:
        with (
            tc.tile_pool(name="const", bufs=1) as constp,
            tc.tile_pool(name="big", bufs=1) as bigp,
            tc.tile_pool(name="stage", bufs=2) as stagep,
            tc.tile_pool(name="strip", bufs=2) as stripp,
            tc.tile_pool(name="ptp", bufs=2) as ptp,
            tc.tile_pool(name="bcast", bufs=2) as bcp,
            tc.tile_pool(name="outs", bufs=2) as outp,
            tc.tile_pool(name="acc", bufs=2) as accp,
            tc.tile_pool(name="small", bufs=2) as smallp,
            tc.tile_pool(name="dram", bufs=2, space="DRAM") as dramp,
            tc.tile_pool(name="mm", bufs=4, space="PSUM") as mmp,
            tc.tile_pool(name="tr", bufs=2, space="PSUM") as trp,
            tc.tile_pool(name="rs", bufs=2, space="PSUM") as rsp,
        ):
            ident = constp.tile([P, P], F32)
            make_identity(nc, ident[:])
            ones = constp.tile([P, 1], BF16)
            nc.vector.memset(ones[:], 1.0)
            x_nat = bigp.tile([P, nst, d], BF16)
            xT = bigp.tile([P, nd, s], F32R)
            w_sb = bigp.tile([P, nd, d], F32R)
            xwT = bigp.tile([P, nd, s], F32R)

            def load_w():
                for kt in range(nd):
                    wst = stagep.tile([P, d], F32, tag="st", name="wst")
                    nc.sync.dma_start(wst[:], w_tiled[:, kt, :])
                    nc.vector.tensor_copy(w_sb[:, kt, :], wst[:])

            def load_tile(st):
                xst = stagep.tile([P, d], F32, tag="st", name="xst")
                nc.sync.dma_start(xst[:], x_tiled[:, st, :])
                # round to bf16 for the PV matmul
                nc.vector.tensor_copy(x_nat[:, st, :], xst[:])
                # xT[p, dt, st*128+q] = x[st*128+q, dt*128+p]
                # 4 transposed blocks land in one PSUM bank -> single copy out
                ps = trp.tile([P, nd, P], F32, tag="tr", name="trps")
                for dt in range(nd):
                    nc.tensor.transpose(
                        ps[:, dt, :], xst[:, dt * P:(dt + 1) * P], ident[:]
                    )
                nc.vector.tensor_copy(xT[:, :, st * P:(st + 1) * P], ps[:])

            def xw_chunk(sb):
                # xwT[e, s-block] = sum_d W[d, e] x[s, d]
                for et in range(nd):
                    ps = mmp.tile([P, SB], F32, tag="mm", name="mmps")
                    for kt in range(nd):
                        nc.tensor.matmul(
                            ps[:],
                            w_sb[:, kt, et * P:(et + 1) * P],
                            xT[:, kt, sb * SB:(sb + 1) * SB],
                            start=(kt == 0),
                            stop=(kt == nd - 1),
                        )
                    nc.vector.tensor_copy(xwT[:, et, sb * SB:(sb + 1) * SB], ps[:])

            for sb in range(nsb):
                for st in range(nss * sb, nss * (sb + 1)):
                    load_tile(st)
                if sb == 0:
                    load_w()
                xw_chunk(sb)

            strips = [None] * nsb
            pts = [None] * nsb
            bcs = [None] * nsb

            def sub_exp(sb, tt):
                """shifted-exp of one staged tile (DVE sub + ScalarE exp),
                plus the zipped rowsum partial accumulate (DVE)."""
                strip = strips[sb]
                nc.vector.tensor_sub(
                    strip[:, tt, :], strip[:, tt, :], bcs[sb][:]
                )
                nc.scalar.activation(
                    pts[sb][:, tt, :],
                    strip[:, tt, :],
                    AF.Exp,
                    scale=SCALE,
                )

            def prod_tiles(sb, prev):
                """prodT tiles of block sb (zipped with sub/exp of `prev` so
                no engine's static stream ever stalls)."""
                strip = stripp.tile([P, nst, SB], F32, tag="strip")
                pts[sb] = ptp.tile([P, nst, SB], BF16, tag="pt", name="pt")
                strips[sb] = strip
                acc = None
                for tt in range(nst):
                    ps = mmp.tile([P, SB], F32, tag="mm")
                    for et in range(nd):
                        nc.tensor.matmul(
                            ps[:],
                            xT[:, et, tt * P:(tt + 1) * P],
                            xwT[:, et, sb * SB:(sb + 1) * SB],
                            start=(et == 0),
                            stop=(et == nd - 1),
                        )
                    # stage raw scores (rounding to f32r, on ScalarE to keep
                    # DVE free) and max-accumulate on DVE
                    nc.scalar.copy(strip[:, tt, :], ps[:])
                    acc_new = accp.tile([P, SB], F32, tag="acc")
                    if acc is None:
                        nc.vector.tensor_copy(acc_new[:], ps[:])
                    else:
                        nc.vector.tensor_max(acc_new[:], ps[:], acc[:])
                    acc = acc_new
                    if prev is not None:
                        sub_exp(prev, tt)
                # row max, replicated across all partitions, on idle GpSimd
                bc = bcp.tile([P, SB], F32, tag="bc", name="bc", bufs=1)
                nc.gpsimd.partition_all_reduce(
                    bc[:], acc[:], channels=P, reduce_op=bass_isa.ReduceOp.max
                )
                bcs[sb] = bc

            def stage_b(sb, zipnext=None):
                """rowsum -> reciprocal -> PV matmul -> normalize -> store."""
                ptt = pts[sb]

                rs_ps = rsp.tile([1, SB], F32, tag="rs", name="rsps")
                for tt in range(nst):
                    nc.tensor.matmul(
                        rs_ps[:],
                        ones[:],
                        ptt[:, tt, :],
                        start=(tt == 0),
                        stop=(tt == nst - 1),
                    )
                    if zipnext is not None:
                        sub_exp(zipnext, tt)
                rs_row = smallp.tile([1, SB], F32, tag="rsrow", name="rsrow")
                nc.vector.tensor_copy(rs_row[:], rs_ps[:])
                rs_dram = dramp.tile([SB], F32, tag="rsdram", name="rsdram")
                nc.sync.dma_start(rs_dram[:], rs_row[:])
                rs_part = smallp.tile([P, nss], F32, tag="rspart", name="rspart")
                nc.sync.dma_start(
                    rs_part[:], rs_dram.rearrange("(f p) -> p f", p=P)
                )
                rs_rec = smallp.tile([P, nss], F32, tag="rsrec", name="rsrec")
                nc.vector.reciprocal(rs_rec[:], rs_part[:])

                # out[s, d] = sum_t P[s, t] x[t, d] ; lhsT = PT (already T!)
                for ss in range(nss):
                    ps = mmp.tile([P, d], F32, tag="mm", name="mmps")
                    for tt in range(nst):
                        nc.tensor.matmul(
                            ps[:],
                            ptt[:, tt, ss * P:(ss + 1) * P],
                            x_nat[:, tt, :],
                            start=(tt == 0),
                            stop=(tt == nst - 1),
                        )
                    ot = outp.tile([P, d], F32, tag="ot", name="ot")
                    nc.vector.tensor_scalar_mul(
                        ot[:], ps[:], rs_rec[:, ss:ss + 1]
                    )
                    nc.sync.dma_start(o_tiled[:, sb * nss + ss, :], ot[:])

            # software pipeline: block sb's sub/exp ops are zipped into the
            # next chunk of PE-heavy work (block sb+1's prodT tiles, or the
            # previous block's rowsum loop) so no static engine stream stalls
            # on the cross-engine max reduction.
            if nsb == 1:
                prod_tiles(0, None)
                for tt in range(nst):
                    sub_exp(0, tt)
                stage_b(0)
            else:
                prod_tiles(0, None)
                prod_tiles(1, 0)
                for sb in range(2, nsb):
                    stage_b(sb - 2)
                    prod_tiles(sb, sb - 1)
                stage_b(nsb - 2, zipnext=nsb - 1)
                stage_b(nsb - 1)

    nc.compile()
    return nc


_NC_CACHE = {}


def _get_nc():
    if "nc" not in _NC_CACHE:
        _NC_CACHE["nc"] = build_nc()
    return _NC_CACHE["nc"]


def kernel(x: np.ndarray, attn_matrix: np.ndarray) -> np.ndarray:
    assert x.shape == (B, S, D) and attn_matrix.shape == (D, D)
    nc = _get_nc()
    w = np.ascontiguousarray(attn_matrix, dtype=np.float32)
    in_maps = [
        {"x": np.ascontiguousarray(x[b], dtype=np.float32), "w": w}
        for b in range(B)
    ]
    res = bass_utils.run_bass_kernel_spmd(nc, in_maps, core_ids=list(range(B)))
    out = np.stack([res.results[b]["o"] for b in range(B)], axis=0)
    return out.astype(np.float32, copy=False)


# revision 28
# speedup vs baseline: 1.4773x; 1.0656x over previous
"""Bass/Tile kernel for BilinearAttentionLayer on 8 NeuronCores.

out[b] = softmax(x[b] @ W @ x[b]^T / sqrt(D)) @ x[b]

Sharding: data-parallel over batch (8 batches -> 8 cores). Each core runs an
identical program on its own batch slice.

Per-core layout: keep the score matrix transposed (scoresT[t, s]) so every
matmul operand is naturally oriented and no score-matrix transposes are
needed:
  xwT   = matmul(lhsT=W,  rhs=xT)       [e, s]
  prodT = matmul(lhsT=xT, rhs=xwT)      [t, s]   (softmax axis = partitions)
  PT    = exp((prodT - rowmax)/sqrt(D))           (numerically safe softmax)
  rowsum= matmul(lhsT=ones, rhs=PT)     [1, s]
  out   = matmul(lhsT=PT, rhs=x_nat)    [s, d]   (lands natural)
The only data transposes are 64 PE transposes of x itself plus tiny
[1,512]<->[128,4] DRAM bounces for the per-row max / rowsum vectors.

Row max (softmax axis = partitions here) is computed as: DVE max-accumulate
across the 16 prodT tiles -> PE transpose of the [128, 512] accumulator ->
DVE free-axis reduce -> broadcast back across partitions with a K=1
ones-matmul -> DVE subtract (in place on the staged prodT strip) -> ScalarE
Exp in place (rounding to f32r for the PE).

Matmul operands are stored as float32r (1 cycle/row on the PE at N=512 vs 4
for float32); producers round to f32r as walrus requires.
"""

import numpy as np

import concourse.bass as bass
import concourse.mybir as mybir
import concourse.tile as tile
from concourse import bacc
from concourse import bass_isa
from concourse import bass_utils
from concourse.masks import make_identity

B = 8
S = 2048
D = 512
P = 128
SB = 512  # s-block width (one fp32 PSUM bank)

F32 = mybir.dt.float32
F32R = mybir.dt.float32r
BF16 = mybir.dt.bfloat16

SCALE = float(1.0 / np.sqrt(np.float64(D)))
AX = mybir.AxisListType
AF = mybir.ActivationFunctionType


def build_nc(s=S, d=D):
    nd = d // P   # d/e tiles of 128
    nst = s // P  # s/t tiles of 128
    nsb = s // SB  # s-blocks
    nss = SB // P  # 128-chunks per s-block

    nc = bacc.Bacc(
        "TRN2",
        target_bir_lowering=False,
        debug=False,
        num_devices=B,
    )
    x_d = nc.dram_tensor("x", [s, d], F32, kind="ExternalInput").ap()
    w_d = nc.dram_tensor("w", [d, d], F32, kind="ExternalInput").ap()
    o_d = nc.dram_tensor("o", [s, d], F32, kind="ExternalOutput").ap()

    x_tiled = x_d.rearrange("(n p) d -> p n d", p=P)  # [128, nst, d]
    w_tiled = w_d.rearrange("(k p) e -> p k e", p=P)  # [128, nd, d]
    o_tiled = o_d.rearrange("(n p) d -> p n d", p=P)

    with tile.TileContext(nc) as tc:
        with (
            tc.tile_pool(name="const", bufs=1) as constp,
            tc.tile_pool(name="big", bufs=1) as bigp,
            tc.tile_pool(name="stage", bufs=3) as stagep,
            tc.tile_pool(name="strip", bufs=2) as stripp,
            tc.tile_pool(name="ptp", bufs=2) as ptp,
            tc.tile_pool(name="bcast", bufs=2) as bcp,
            tc.tile_pool(name="outs", bufs=2) as outp,
            tc.tile_pool(name="acc", bufs=2) as accp,
            tc.tile_pool(name="small", bufs=2) as smallp,
            tc.tile_pool(name="dram", bufs=2, space="DRAM") as dramp,
            tc.tile_pool(name="mm", bufs=6, space="PSUM") as mmp,
            tc.tile_pool(name="tr", bufs=2, space="PSUM") as trp,
        ):
            ident = constp.tile([P, P], F32)
            make_identity(nc, ident[:])
            ones = constp.tile([P, 1], BF16)
            nc.vector.memset(ones[:], 1.0)
            x_nat = bigp.tile([P, nst, d], BF16)
            xT = bigp.tile([P, nd, s], F32R)
            w_sb = bigp.tile([P, nd, d], F32R)
            xwT = bigp.tile([P, nd, s], F32R)

            def load_w():
                for kt in range(nd):
                    wst = stagep.tile([P, d], F32, tag="st", name="wst")
                    nc.sync.dma_start(wst[:], w_tiled[:, kt, :])
                    nc.vector.tensor_copy(w_sb[:, kt, :], wst[:])

            def load_tile(st):
                xst = stagep.tile([P, d], F32, tag="st", name="xst")
                nc.sync.dma_start(xst[:], x_tiled[:, st, :])
                # round to bf16 for the PV matmul
                nc.vector.tensor_copy(x_nat[:, st, :], xst[:])
                # xT[p, dt, st*128+q] = x[st*128+q, dt*128+p]
                # 4 transposed blocks land in one PSUM bank -> single copy out
                ps = trp.tile([P, nd, P], F32, tag="tr", name="trps")
                for dt in range(nd):
                    nc.tensor.transpose(
                        ps[:, dt, :], xst[:, dt * P:(dt + 1) * P], ident[:]
                    )
                nc.vector.tensor_copy(xT[:, :, st * P:(st + 1) * P], ps[:])

            def xw_chunk(sb):
                # xwT[e, s-block] = sum_d W[d, e] x[s, d]
                for et in range(nd):
                    ps = mmp.tile([P, SB], F32, tag="mm", name="mmps")
                    for kt in range(nd):
                        nc.tensor.matmul(
                            ps[:],
                            w_sb[:, kt, et * P:(et + 1) * P],
                            xT[:, kt, sb * SB:(sb + 1) * SB],
                            start=(kt == 0),
                            stop=(kt == nd - 1),
                        )
                    nc.vector.tensor_copy(xwT[:, et, sb * SB:(sb + 1) * SB], ps[:])

            for sb in range(nsb):
                for st in range(nss * sb, nss * (sb + 1)):
                    load_tile(st)
                if sb == 0:
                    load_w()
                xw_chunk(sb)

            strips = [None] * nsb
            pts = [None] * nsb
            bcs = [None] * nsb

            def sub_exp(sb, tt):
                """shifted-exp of one staged tile (DVE sub + ScalarE exp),
                plus the zipped rowsum partial accumulate (DVE)."""
                strip = strips[sb]
                nc.vector.tensor_sub(
                    strip[:, tt, :], strip[:, tt, :], bcs[sb][:]
                )
                nc.scalar.activation(
                    pts[sb][:, tt, :],
                    strip[:, tt, :],
                    AF.Exp,
                    scale=SCALE,
                )

            def prod_tiles(sb, prev):
                """prodT tiles of block sb (zipped with sub/exp of `prev` so
                no engine's static stream ever stalls)."""
                strip = stripp.tile([P, nst, SB], F32, tag="strip")
                pts[sb] = ptp.tile([P, nst, SB], BF16, tag="pt", name="pt")
                strips[sb] = strip
                acc = None
                for tt in range(nst):
                    ps = mmp.tile([P, SB], F32, tag="mm")
                    for et in range(nd):
                        nc.tensor.matmul(
                            ps[:],
                            xT[:, et, tt * P:(tt + 1) * P],
                            xwT[:, et, sb * SB:(sb + 1) * SB],
                            start=(et == 0),
                            stop=(et == nd - 1),
                        )
                    # stage raw scores (rounding to f32r, on ScalarE to keep
                    # DVE free) and max-accumulate on DVE
                    nc.scalar.copy(strip[:, tt, :], ps[:])
                    acc_new = accp.tile([P, SB], F32, tag="acc")
                    if acc is None:
                        nc.vector.tensor_copy(acc_new[:], ps[:])
                    else:
                        nc.vector.tensor_max(acc_new[:], ps[:], acc[:])
                    acc = acc_new
                    if prev is not None:
                        sub_exp(prev, tt)
                # row max, replicated across all partitions, on idle GpSimd
                bc = bcp.tile([P, SB], F32, tag="bc", name="bc", bufs=1)
                nc.gpsimd.partition_all_reduce(
                    bc[:], acc[:], channels=P, reduce_op=bass_isa.ReduceOp.max
                )
                bcs[sb] = bc

            def stage_b(sb, zipnext=None):
                """rowsum -> reciprocal -> PV matmul -> normalize -> store."""
                ptt = pts[sb]

                rs_ps = trp.tile([1, SB], F32, tag="tr", name="rsps")
                for tt in range(nst):
                    nc.tensor.matmul(
                        rs_ps[:],
                        ones[:],
                        ptt[:, tt, :],
                        start=(tt == 0),
                        stop=(tt == nst - 1),
                    )
                    if zipnext is not None:
                        sub_exp(zipnext, tt)
                rs_row = smallp.tile([1, SB], F32, tag="rsrow", name="rsrow")
                nc.vector.tensor_copy(rs_row[:], rs_ps[:])
                rs_dram = dramp.tile([SB], F32, tag="rsdram", name="rsdram")
                nc.sync.dma_start(rs_dram[:], rs_row[:])
                rs_part = smallp.tile([P, nss], F32, tag="rspart", name="rspart")
                nc.sync.dma_start(
                    rs_part[:], rs_dram.rearrange("(f p) -> p f", p=P)
                )
                rs_rec = smallp.tile([P, nss], F32, tag="rsrec", name="rsrec")
                nc.vector.reciprocal(rs_rec[:], rs_part[:])

                # out[s, d] = sum_t P[s, t] x[t, d] ; lhsT = PT (already T!)
                for ss in range(nss):
                    ps = mmp.tile([P, d], F32, tag="mm", name="mmps")
                    for tt in range(nst):
                        nc.tensor.matmul(
                            ps[:],
                            ptt[:, tt, ss * P:(ss + 1) * P],
                            x_nat[:, tt, :],
                            start=(tt == 0),
                            stop=(tt == nst - 1),
                        )
                    ot = outp.tile([P, d], F32, tag="ot", name="ot")
                    nc.vector.tensor_scalar_mul(
                        ot[:], ps[:], rs_rec[:, ss:ss + 1]
                    )
                    nc.sync.dma_start(o_tiled[:, sb * nss + ss, :], ot[:])

            # software pipeline: block sb's sub/exp ops are zipped into the
            # next chunk of PE-heavy work (block sb+1's prodT tiles, or the
            # previous block's rowsum loop) so no static engine stream stalls
            # on the cross-engine max reduction.
            if nsb == 1:
                prod_tiles(0, None)
                for tt in range(nst):
                    sub_exp(0, tt)
                stage_b(0)
            else:
                prod_tiles(0, None)
                prod_tiles(1, 0)
                for sb in range(2, nsb):
                    stage_b(sb - 2)
                    prod_tiles(sb, sb - 1)
                stage_b(nsb - 2, zipnext=nsb - 1)
                stage_b(nsb - 1)

    nc.compile()
    return nc


_NC_CACHE = {}


def _get_nc():
    if "nc" not in _NC_CACHE:
        _NC_CACHE["nc"] = build_nc()
    return _NC_CACHE["nc"]


def kernel(x: np.ndarray, attn_matrix: np.ndarray) -> np.ndarray:
    assert x.shape == (B, S, D) and attn_matrix.shape == (D, D)
    nc = _get_nc()
    w = np.ascontiguousarray(attn_matrix, dtype=np.float32)
    in_maps = [
        {"x": np.ascontiguousarray(x[b], dtype=np.float32), "w": w}
        for b in range(B)
    ]
    res = bass_utils.run_bass_kernel_spmd(nc, in_maps, core_ids=list(range(B)))
    out = np.stack([res.results[b]["o"] for b in range(B)], axis=0)
    return out.astype(np.float32, copy=False)
